# revision 9
# baseline (speedup 1.0000x reference)
"""Two-layer GCN (PyG GCNConv x2 + rrelu) on 8 Trainium2 NeuronCores.

Math: with A = adjacency-with-multiplicity + I (self loops), deg = in-degree
(including the self loop), dinv = deg^-1/2:
    z1[v] = dinv[v] * (sum_{u->v} dinv[u]*x[u]) @ W1 + b1
    g[u]  = dinv[u] * rrelu(z1[u])                      (dinv pre-folded for L2)
    z2[v] = dinv[v] * (sum_{u->v} g[u]) @ W2 + b2
Aggregation is linear, so the dense W matmul is applied post-aggregation on
the [128, 128] per-destination-block aggregate -- 128x less PE work than
transforming every edge message.

Sharding: destinations are range-sharded across the 8 cores (12544 each).
Every core keeps a replicated (dinv-prescaled, bf16) source-feature table in
its own HBM and fetches the source rows of its edges with dma_gather (int16
indices -> four even source windows).  SWDGE descriptor generation costs
~3ns/index of Pool-engine time (linear; measured), which makes total gather
slots the dominant cost -- so edges are packed TIGHTLY: per (superblock of 7
dest blocks, window) all edges are concatenated block-major with a single
duplicate-padded tail rounded to 128, instead of per-(block, window) padded
buckets (251k -> ~208k slots/core/layer).  Gather columns may straddle two
dest blocks; each block's TensorEngine scatter-reduce (matmul with one-hot
selectors Sel[e, dest] = (d[e] == dest), generated on-device by is_equal
with broadcast operand, d = -1 for foreign/pad slots) walks the union (over
cores, for SPMD uniformity) of the column spans its edges occupy.  Gather
calls are capped at 1024 indices (the stable SWDGE ring limit) and round-
robin the 4 SWDGE queues.  Self-loop contributions bypass the gather: their
source rows are contiguous, so a plain DMA + identity matmul adds them.
Two NEFF dispatches (layer 1, layer 2); the host transposes/concats
activations between them.

The harness calls kernel(**inputs) with full inputs; index bucketing,
program build, compile, SPMD run on cores 0-7 and unshard all happen here.
"""

import sys

for _p in ("/opt/trn_rl_repo",):
    if _p not in sys.path:
        sys.path.insert(0, _p)

import numpy as np
import ml_dtypes

import concourse.bacc as bacc
import concourse.bass as bass
import concourse.mybir as mybir
import concourse.tile as tile
from concourse.bass_utils import run_bass_kernel_spmd

P = 128  # partition width == dest block width == feature width
RRELU_SLOPE = (1.0 / 8.0 + 1.0 / 3.0) / 2.0
MAX_IDX_PER_CALL = 1024   # SWDGE ring: >1024-index gathers fault at depth
G = 16                    # sel-gen group width (columns per is_equal)


class Cfg:
    def __init__(self, n_nodes, n_cores, blocks_per_core, superblock, in_f,
                 out1_f, out2_f, src_window):
        self.n_nodes = n_nodes
        self.n_cores = n_cores
        self.bpc = blocks_per_core            # dest blocks per core
        self.sb = superblock                  # blocks per superblock
        assert blocks_per_core % superblock == 0
        self.sb_count = blocks_per_core // superblock
        self.in_f = in_f
        self.out1_f = out1_f
        self.out2_f = out2_f
        self.src_window = src_window          # int16 gather range per window
        self.nodes_per_core = blocks_per_core * P
        self.n_pad = n_cores * self.nodes_per_core
        assert self.n_pad >= n_nodes
        assert src_window % P == 0 and src_window <= 32768
        self.n_chunks = -(-self.n_pad // src_window)
        self.tab_rows = self.n_chunks * src_window


FULL = Cfg(n_nodes=100000, n_cores=8, blocks_per_core=98, superblock=7,
           in_f=128, out1_f=128, out2_f=64, src_window=25088)


# --------------------------------------------------------------------------
# host-side index preprocessing
# --------------------------------------------------------------------------

def preprocess(edge_index, cfg):
    """Bucket edges by (superblock, window) concatenated block-major; self
    loops are handled separately on-device.

    Returns a plan shared by all cores (SPMD-uniform):
      cols[s][k]   -- gather columns per (superblock, window)
      spans[s][k][b7] = (LO, HI) -- union column span of block b7's edges
    and per-core tensors: idx_tab (gather indices, window-relative, wrapped
    16-partition-per-call), d_tab (dest-local index per (block-span column,
    slot), -1 for foreign/pad), dinv_sl.
    """
    row = edge_index[0].astype(np.int64)
    col = edge_index[1].astype(np.int64)
    n = cfg.n_nodes
    C, SB, NK, B7 = cfg.n_cores, cfg.sb_count, cfg.n_chunks, cfg.sb

    deg = np.bincount(col, minlength=cfg.n_pad).astype(np.float64) + 1.0
    dinv = (1.0 / np.sqrt(deg)).astype(np.float32)
    dinv[n:] = 1.0

    blk = col >> 7                      # global dest block
    core = blk // cfg.bpc
    b_loc = blk % cfg.bpc
    s_idx = b_loc // B7
    b7 = b_loc % B7
    k = row // cfg.src_window

    # sort edges by (core, s, k, b7); ties keep source order (irrelevant)
    order = np.lexsort((row, b7, k, s_idx, core))
    row, col, core, s_idx, k, b7 = (a[order] for a in (row, col, core, s_idx, k, b7))

    counts = np.zeros((C, SB, NK, B7), dtype=np.int64)
    np.add.at(counts, (core, s_idx, k, b7), 1)
    # per-(c,s,k): block start positions within the concat; totals
    b7_start = np.cumsum(counts, axis=3) - counts          # exclusive prefix
    totals = counts.sum(axis=3)                            # [C, SB, NK]
    n_sk = totals.max(axis=0)                              # [SB, NK]
    cols = -(-n_sk // P).astype(np.int64)                  # pad to full cols
    n_sk = cols * P

    # union spans over cores (empty-count cores excluded from min/max)
    pos_lo = b7_start
    pos_hi = b7_start + counts
    col_lo = pos_lo // P
    col_hi = (np.maximum(pos_hi, 1) - 1) // P
    has = counts > 0
    big = np.where(has, col_lo, np.iinfo(np.int64).max)
    LO = big.min(axis=0)                                   # [SB, NK, B7]
    small = np.where(has, col_hi, -1)
    HI = small.max(axis=0)
    any_edges = has.any(axis=0)
    spans = [[[(int(LO[s, kk, b]), int(HI[s, kk, b])) if any_edges[s, kk, b]
               else None for b in range(B7)]
              for kk in range(NK)] for s in range(SB)]

    # d-tab columns per block: block-major contiguous runs
    dcols_per_block = [[sum((spans[s][kk][b][1] - spans[s][kk][b][0] + 1)
                            if spans[s][kk][b] else 0 for kk in range(NK))
                        for b in range(B7)] for s in range(SB)]
    d_total = sum(sum(r) for r in dcols_per_block)

    # edge start offset per (c, s, k) in the sorted edge array
    csk_counts = totals.reshape(C, SB * NK)
    csk_start = np.zeros(C * SB * NK + 1, dtype=np.int64)
    np.cumsum(csk_counts.reshape(-1), out=csk_start[1:])

    per_core = []
    for c in range(C):
        idx_parts = []
        d_tab = np.full((P, d_total), -1.0, dtype=np.float64)
        dcol0 = 0
        for s in range(SB):
            # build the (s, k) index segments
            seg_slots = {}
            for kk in range(NK):
                lo = csk_start[(c * SB + s) * NK + kk]
                hi = csk_start[(c * SB + s) * NK + kk + 1]
                cnt = hi - lo
                Nsk = int(n_sk[s, kk])
                assert cnt <= Nsk
                seg = np.zeros(Nsk, dtype=np.int64)
                seg[:cnt] = row[lo:hi] - kk * cfg.src_window
                if cnt < Nsk:
                    seg[cnt:] = seg[0] if cnt > 0 else 0
                assert seg.min() >= 0 and seg.max() < cfg.src_window
                # wrap per subcall of <=1024
                o = 0
                while o < Nsk:
                    m = min(MAX_IDX_PER_CALL, Nsk - o)
                    idx_parts.append(seg[o:o + m].astype(np.int16))
                    o += m
                # dest-local values per slot (-1 for pad)
                dv = np.full(Nsk, -1.0)
                dv[:cnt] = (col[lo:hi] % P).astype(np.float64)
                bv = np.full(Nsk, -1, dtype=np.int64)
                bv[:cnt] = b7[lo:hi]
                seg_slots[kk] = (dv, bv)
            # d_tab columns, block-major
            for b in range(B7):
                for kk in range(NK):
                    sp = spans[s][kk][b]
                    if sp is None:
                        continue
                    dv, bv = seg_slots[kk]
                    for cc in range(sp[0], sp[1] + 1):
                        sl = slice(cc * P, (cc + 1) * P)
                        dcol = np.where(bv[sl] == b, dv[sl], -1.0)
                        d_tab[:, dcol0] = dcol
                        dcol0 += 1
        assert dcol0 == d_total
        idx_flat = [a.reshape(-1, 16).T for a in idx_parts]
        idx_tab = np.tile(np.concatenate(idx_flat, axis=1), (8, 1))
        per_core.append({
            "idx_tab": np.ascontiguousarray(idx_tab),
            "d_tab": np.ascontiguousarray(d_tab.astype(ml_dtypes.bfloat16)),
            "dinv_sl": np.ascontiguousarray(
                dinv[c * cfg.nodes_per_core:(c + 1) * cfg.nodes_per_core]
            ).reshape(1, -1),
        })

    plan = {"cols": cols, "n_sk": n_sk, "spans": spans,
            "dcols_per_block": dcols_per_block, "d_total": d_total}
    return {"plan": plan, "dinv": dinv, "per_core": per_core}


# --------------------------------------------------------------------------
# bass program (one GCN layer, SPMD across cores; all data via inputs)
# --------------------------------------------------------------------------

def build_layer_program(cfg, plan, layer):
    """layer=1: out = bf16 gs1T [128, nodes_per_core]  (dinv*rrelu(z1), F-major)
       layer=2: out = f32  z2T  [out2_f, nodes_per_core]"""
    cols = plan["cols"]                  # [SB, NK]
    n_sk = plan["n_sk"]
    spans = plan["spans"]
    d_total = plan["d_total"]
    SB, NK, B7 = cfg.sb_count, cfg.n_chunks, cfg.sb
    out_f = cfg.out1_f if layer == 1 else cfg.out2_f
    out_dt = mybir.dt.bfloat16 if layer == 1 else mybir.dt.float32
    idx_cols_s = [int(n_sk[s].sum()) // 16 for s in range(SB)]
    msg_cols_s = [int(cols[s].sum()) for s in range(SB)]
    max_msg_cols = max(msg_cols_s)
    max_idx_cols = max(idx_cols_s)

    nc = bacc.Bacc("TRN2", target_bir_lowering=False, debug=False,
                   num_devices=cfg.n_cores, num_swdge_queues=2)
    dt = mybir.dt
    src_tab = nc.dram_tensor("src_tab", [cfg.tab_rows, P], dt.bfloat16,
                             kind="ExternalInput")
    w_in = nc.dram_tensor("w", [P, out_f], dt.bfloat16, kind="ExternalInput")
    bias_in = nc.dram_tensor("bias", [out_f, 1], dt.float32, kind="ExternalInput")
    dinv_in = nc.dram_tensor("dinv_sl", [1, cfg.nodes_per_core], dt.float32,
                             kind="ExternalInput")
    idx_in = nc.dram_tensor("idx_tab", [P, sum(idx_cols_s)], dt.int16,
                            kind="ExternalInput")
    d_in = nc.dram_tensor("d_tab", [P, d_total], dt.bfloat16,
                          kind="ExternalInput")
    iota_in = nc.dram_tensor("iota", [P, G * P], dt.bfloat16, kind="ExternalInput")
    ident_in = nc.dram_tensor("ident", [P, P], dt.bfloat16, kind="ExternalInput")
    ones_in = nc.dram_tensor("ones", [1, P], dt.float32, kind="ExternalInput")
    out_t = nc.dram_tensor("out_t", [out_f, cfg.nodes_per_core], out_dt,
                           kind="ExternalOutput")
    # per-core self-loop source rows, staged by the host (node-major slice of
    # src_tab rows owned by this core; avoids needing the core id on device)
    self_in = nc.dram_tensor("self_rows", [cfg.nodes_per_core, P], dt.bfloat16,
                             kind="ExternalInput")

    with tile.TileContext(nc) as tc:
        with (
            tc.tile_pool(name="const", bufs=1) as const_pool,
            tc.tile_pool(name="idx", bufs=3) as idx_pool,
            tc.tile_pool(name="msg", bufs=3) as msg_pool,
            tc.tile_pool(name="selfp", bufs=2) as self_pool,
            tc.tile_pool(name="sel", bufs=6) as sel_pool,
            tc.tile_pool(name="aggsb", bufs=3) as aggsb_pool,
            tc.tile_pool(name="tmp", bufs=3) as tmp_pool,
            tc.tile_pool(name="outsb", bufs=2) as out_pool,
            tc.tile_pool(name="psA", bufs=2, space="PSUM") as agg_psum,
            tc.tile_pool(name="psZ", bufs=2, space="PSUM") as z_psum,
            tc.tile_pool(name="psD", bufs=2, space="PSUM") as d_psum,
        ):
            w_sb = const_pool.tile([P, out_f], dt.bfloat16)
            nc.scalar.dma_start(out=w_sb[:], in_=w_in[:])
            bias_sb = const_pool.tile([out_f, 1], dt.float32)
            nc.scalar.dma_start(out=bias_sb[:], in_=bias_in[:])
            dinv_sb = const_pool.tile([1, cfg.nodes_per_core], dt.float32)
            nc.scalar.dma_start(out=dinv_sb[:], in_=dinv_in[:])
            iota_sb = const_pool.tile([P, G * P], dt.bfloat16)
            nc.scalar.dma_start(out=iota_sb[:], in_=iota_in[:])
            ident_sb = const_pool.tile([P, P], dt.bfloat16)
            nc.scalar.dma_start(out=ident_sb[:], in_=ident_in[:])
            ones_sb = const_pool.tile([1, P], dt.float32)
            nc.scalar.dma_start(out=ones_sb[:], in_=ones_in[:])
            d_sb = const_pool.tile([P, d_total], dt.bfloat16)
            nc.scalar.dma_start(out=d_sb[:], in_=d_in[:])

            self_view = self_in.rearrange("(s b p) f -> s p b f",
                                          p=P, b=cfg.sb)

            idx_off = 0       # running offset into idx_in (free dim)
            dcol0 = 0         # running offset into d_sb
            qq = 0            # SWDGE queue round-robin
            for s in range(SB):
                idx_sb = idx_pool.tile([P, max_idx_cols], dt.int16,
                                       tag="idx")
                nc.sync.dma_start(
                    out=idx_sb[:, :idx_cols_s[s]],
                    in_=idx_in[:, idx_off:idx_off + idx_cols_s[s]])
                idx_off += idx_cols_s[s]
                # contiguous self-loop rows for this superblock
                selfs = self_pool.tile([P, cfg.sb, P], dt.bfloat16)
                nc.sync.dma_start(out=selfs[:], in_=self_view[s])

                msg = msg_pool.tile([P, max_msg_cols, P], dt.bfloat16,
                                    tag="msg")
                ioff = 0      # within this superblock's idx slice (int16 cols)
                mbase = []    # msg column base per window
                mb = 0
                for kk in range(NK):
                    mbase.append(mb)
                    Nsk = int(n_sk[s, kk])
                    o = 0
                    while o < Nsk:
                        m = min(MAX_IDX_PER_CALL, Nsk - o)
                        nc.gpsimd.dma_gather(
                            msg[:, mb + o // P:mb + (o + m) // P, :],
                            src_tab[kk * cfg.src_window:
                                    (kk + 1) * cfg.src_window, :],
                            idx_sb[:, ioff:ioff + m // 16],
                            m, m, P,
                            queue_num=qq % 2,
                        )
                        qq += 1
                        o += m
                        ioff += m // 16
                    mb += int(cols[s, kk])

                out_sb = out_pool.tile([out_f, cfg.sb * P], out_dt)
                for b7 in range(B7):
                    b_loc = s * cfg.sb + b7
                    # this block's matmul columns: union spans, block-major
                    mcols = []
                    for kk in range(NK):
                        sp = spans[s][kk][b7]
                        if sp is None:
                            continue
                        for cc in range(sp[0], sp[1] + 1):
                            mcols.append(mbase[kk] + cc)
                    nsel = len(mcols)
                    sels = []
                    done = 0
                    while done < nsel:
                        g = min(G, nsel - done)
                        sel = sel_pool.tile([P, G * P], dt.bfloat16)
                        nc.vector.tensor_tensor(
                            sel[:, :g * P],
                            iota_sb[:, :g * P],
                            d_sb[:, dcol0 + done:dcol0 + done + g]
                                .to_broadcast([P, g, P]),
                            mybir.AluOpType.is_equal,
                        )
                        sels.extend((sel, j) for j in range(g))
                        done += g
                    dcol0 += nsel

                    agg = agg_psum.tile([P, P], dt.float32)
                    for ci, mcol in enumerate(mcols):
                        sel, j = sels[ci]
                        nc.tensor.matmul(
                            agg[:],
                            lhsT=msg[:, mcol, :],
                            rhs=sel[:, j * P:(j + 1) * P],
                            start=(ci == 0), stop=False,
                        )
                    # self-loop contribution: aggT += selfs[:, b7, :]^T
                    nc.tensor.matmul(
                        agg[:], lhsT=selfs[:, b7, :], rhs=ident_sb[:],
                        start=(nsel == 0), stop=True)

                    # dinv broadcast tile for this block (rank-1 matmul into
                    # psum, then to SBUF via the idle ScalarEngine -- DVE may
                    # read only one PSUM operand and zps is already PSUM)
                    dps = d_psum.tile([P, P], dt.float32)
                    nc.tensor.matmul(
                        dps[:], lhsT=ones_sb[:],
                        rhs=dinv_sb[:, b_loc * P:(b_loc + 1) * P],
                        start=True, stop=True)
                    dbc = aggsb_pool.tile([P, P], dt.float32, tag="dbc")
                    nc.scalar.copy(dbc[:], dps[:])

                    aggsb = aggsb_pool.tile([P, P], dt.bfloat16, tag="aggsb")
                    nc.scalar.copy(aggsb[:], agg[:])

                    zps = z_psum.tile([out_f, P], dt.float32)
                    nc.tensor.matmul(zps[:], lhsT=w_sb[:], rhs=aggsb[:],
                                     start=True, stop=True)

                    # t1 = dinv[v] * (agg @ W) on DVE (free-dim varying dinv);
                    # then +bias (per-partition AP) and rrelu fused on the
                    # otherwise-idle Activation engine.
                    o_sl = out_sb[:, b7 * P:(b7 + 1) * P]
                    if layer == 1:
                        t1 = tmp_pool.tile([P, P], dt.float32, tag="t1")
                        nc.vector.tensor_tensor(t1[:], zps[:], dbc[:],
                                                mybir.AluOpType.mult)
                        rr = tmp_pool.tile([P, P], dt.float32, tag="rr")
                        nc.scalar.activation(
                            rr[:], t1[:], mybir.ActivationFunctionType.Prelu,
                            bias=bias_sb[:, 0:1], alpha=float(RRELU_SLOPE))
                        nc.vector.tensor_tensor(o_sl, rr[:], dbc[:],
                                                mybir.AluOpType.mult)
                    else:
                        t1 = tmp_pool.tile([out_f, P], dt.float32, tag="t1")
                        nc.vector.tensor_tensor(t1[:], zps[:], dbc[:out_f, :],
                                                mybir.AluOpType.mult)
                        nc.scalar.activation(
                            o_sl, t1[:], mybir.ActivationFunctionType.Identity,
                            bias=bias_sb[:, 0:1])

                nc.sync.dma_start(
                    out=out_t[:, s * cfg.sb * P:(s + 1) * cfg.sb * P],
                    in_=out_sb[:])

    nc.compile()
    return nc


# --------------------------------------------------------------------------
# orchestration
# --------------------------------------------------------------------------

def _iota_tile():
    return np.tile(np.arange(P, dtype=np.float32), G)[None, :].repeat(P, 0).astype(ml_dtypes.bfloat16)


def _run_gcn(x, edge_index, W1, b1, W2, b2, cfg, runner=None, want_times=False):
    """Shared driver; runner(nc, in_maps) -> list of per-core output dicts."""
    meta = preprocess(np.asarray(edge_index), cfg)
    dinv = meta["dinv"]
    npc = cfg.nodes_per_core

    if runner is None:
        times = []

        def runner(nc, in_maps):
            r = run_bass_kernel_spmd(nc, in_maps, core_ids=list(range(cfg.n_cores)),
                                     trace=want_times)
            if want_times:
                times.append(r.exec_time_ns)
            return r.results
    else:
        times = None

    x = np.asarray(x, dtype=np.float32)
    xs = np.zeros((cfg.tab_rows, P), dtype=ml_dtypes.bfloat16)
    xs[:cfg.n_nodes] = (x * dinv[:cfg.n_nodes, None]).astype(ml_dtypes.bfloat16)

    iota = _iota_tile()
    ident = np.eye(P, dtype=np.float32).astype(ml_dtypes.bfloat16)
    ones = np.ones((1, P), np.float32)
    w1 = np.asarray(W1, np.float32).astype(ml_dtypes.bfloat16)
    w2 = np.asarray(W2, np.float32).astype(ml_dtypes.bfloat16)
    b1c = np.asarray(b1, np.float32).reshape(-1, 1)
    b2c = np.asarray(b2, np.float32).reshape(-1, 1)

    nc1 = build_layer_program(cfg, meta["plan"], layer=1)
    in_maps = [
        {"src_tab": xs, "w": w1, "bias": b1c, "iota": iota, "ident": ident,
         "ones": ones,
         "self_rows": np.ascontiguousarray(xs[c * npc:(c + 1) * npc]),
         **{k: pc[k] for k in ("idx_tab", "d_tab", "dinv_sl")}}
        for c, pc in enumerate(meta["per_core"])
    ]
    res1 = runner(nc1, in_maps)

    gs = np.zeros((cfg.tab_rows, P), dtype=ml_dtypes.bfloat16)
    for c in range(cfg.n_cores):
        gs[c * npc:(c + 1) * npc] = res1[c]["out_t"].T

    nc2 = build_layer_program(cfg, meta["plan"], layer=2)
    for c in range(cfg.n_cores):
        in_maps[c] = dict(in_maps[c])
        in_maps[c]["src_tab"] = gs
        in_maps[c]["self_rows"] = np.ascontiguousarray(gs[c * npc:(c + 1) * npc])
        in_maps[c]["w"] = w2
        in_maps[c]["bias"] = b2c
    res2 = runner(nc2, in_maps)

    out = np.zeros((cfg.n_pad, cfg.out2_f), dtype=np.float32)
    for c in range(cfg.n_cores):
        out[c * npc:(c + 1) * npc] = res2[c]["out_t"].T
    out = out[:cfg.n_nodes]
    if want_times and times is not None:
        return out, times
    return out


def kernel(x, edge_index, W1, b1, W2, b2):
    return _run_gcn(x, edge_index, W1, b1, W2, b2, FULL)
